# revision 19
# baseline (speedup 1.0000x reference)
"""Trainium2 Bass kernel for nn_DiffusionBlock (anisotropic diffusion step).

Math (per batch, channel image; s = tau*hx^2, hx = grad kernel tap):
  X[i,j] = u[i,j+1]-u[i,j] (0 at j=W-1),  Y[i,j] = u[i+1,j]-u[i,j] (0 at i=H-1)
  XP/YP  = edge-pad(X/Y) on the (H+2, W+2) grid
  F = a*XP + b*YP,  G = b*XP + c*YP              (padded grid)
  out[i,j] = u[i,j] + s*(F[i+1,j+1]-F[i+1,j] + G[i+1,j+1]-G[i,j+1])

Per-core layout (pure batch data-parallel across 8 cores, 1 batch each):
row-tiles of R=126 output rows. SBUF partition q holds:
  U[q]     = u row r0-1+q (edge-clamped)         [R+2, W]
  A/B/C[q] = a/b/c row r0+q                      [R+1, W+1]
  XT[q]    = X row r0-1+q (DVE free-dim diff)    [R+1, W]
  YT[q]    = Y row r0-1+q (PE bidiagonal matmul) [R+1, W]  (PSUM)
Products (DVE, all partition-aligned, PSUM read for YT):
  PA = A*XTc, PB1 = B*YTc, PB2 = B*XTc, PC = C*YTc   (c = col-clamped shift)
PE assembles the output in PSUM with constant weight matrices (partition
shifts, signs and the scale s all folded into the weights):
  OUT[p] = U[p+1] + s*((PA+PB1)[p+1]@j+1 - (PA+PB1)[p+1]@j) + Wg@(PB2+PC)
ACT copies PSUM->SBUF, DMA stores.

All DRAM tensors are bf16, coefficients fp8-e4m3 (host casts inputs, upcasts the output): cuts
HBM traffic, which is the bottleneck (DMA cost here ~ 1.45 ns per
partition-byte for loads, ~2.7 for stores, serialized per core; 128- or
<=126-partition single-run-per-partition DMAs only -- 127 partitions or
multi-run APs cost ~4x). Coefficient loads are 128 rows (not ka=127) to
dodge the 127-partition penalty. PSUM accumulation stays fp32.
"""

import numpy as np

# Problem geometry (hardcoded per harness contract).
N_CORES = 8
N_CH = 2
H = 1024
W = 1024
R = 126       # output rows per tile
CHUNK = 512   # matmul free-dim chunk (= one PSUM bank of fp32)

_W_NAMES = ("wu", "wsp", "wsn", "wg", "my", "myf", "myl", "myfl")


def _host_weights(s: float, rt_last: int):
    """Constant PE weight matrices, packed [128, 8*128] bf16.

    matmul(out, lhsT, rhs): out[p, n] = sum_k lhsT[k, p] * rhs[k, n]
    """
    import ml_dtypes

    k = np.arange(128)[:, None]
    p = np.arange(128)[None, :]
    sf = np.float32(s)
    wu = (k == p + 1).astype(np.float32)            # out[p] += U[p+1]
    wsp = sf * (k == p + 1)                         # out[p] += s * x[p+1]
    wsn = -sf * (k == p + 1)                        # out[p] -= s * x[p+1]
    wg = sf * (k == p + 1) - sf * (k == p)
    my = ((k == p + 1).astype(np.float32) - (k == p))  # YT[q] = U[q+1]-U[q]
    myf = my.copy()                                 # first tile: YT[0] = U[2]-U[1]
    myf[:, 0] = 0.0
    myf[2, 0] = 1.0
    myf[1, 0] = -1.0
    myl = my.copy()                                 # last tile: YT[rt] = 0
    myl[:, rt_last] = 0.0
    myfl = myf.copy()
    myfl[:, rt_last] = 0.0
    mats = {"wu": wu, "wsp": wsp, "wsn": wsn, "wg": wg,
            "my": my, "myf": myf, "myl": myl, "myfl": myfl}
    return np.ascontiguousarray(
        np.concatenate([mats[n] for n in _W_NAMES], axis=1)
    ).astype(ml_dtypes.bfloat16)


def _build_nc(n_ch: int, h: int, w: int, r: int, chunk: int, reps: int = 1, mode: str = "full"):
    import concourse.bacc as bacc
    import concourse.mybir as mybir
    import concourse.tile as tile

    f32 = mybir.dt.float32
    bf16 = mybir.dt.bfloat16

    nc = bacc.Bacc()
    # per padded-grid row r: u row clamp(r-1,0,h-1), then a/b/c rows r,
    # all bf16 -- so each tile needs ONE input DMA and the first-tile
    # top-clamp row comes packed from the host. bf16 (not fp8) coefficients
    # keep the DVE tensor_tensor ops on the 2x all-SBUF-bf16 path.
    row_e = w + 3 * (w + 2)
    uabc_d = nc.dram_tensor(
        "uabc", [n_ch, h + 2, row_e], bf16, kind="ExternalInput")
    wts_d = nc.dram_tensor("wts", [128, len(_W_NAMES) * 128], bf16, kind="ExternalInput")
    out_d = nc.dram_tensor("out", [n_ch, h, w], bf16, kind="ExternalOutput")

    tiles = [(r0, min(r, h - r0)) for r0 in range(0, h, r)]
    if mode == "min":
        with tile.TileContext(nc) as tc:
            with tc.tile_pool(name="io", bufs=1) as io:
                t = io.tile([1, 16], bf16, tag="t")
                nc.sync.dma_start(t[0:1, :], uabc_d[0, 0:1, 0:16])
                nc.sync.dma_start(out_d[0, 0:1, 0:16], t[0:1, :])
        nc.compile()
        return nc

    with tile.TileContext(nc) as tc:
        with (
            tc.tile_pool(name="wpool", bufs=1) as wpool,
            tc.tile_pool(name="io", bufs=4) as io,
            tc.tile_pool(name="tmp", bufs=3) as tmp,
            tc.tile_pool(name="psum", bufs=2, space="PSUM") as psum,
        ):
            # one DMA for all weights, then a barrier so no later instruction
            # ever waits on this DMA (matmul sync-wait slots are scarce)
            w_all = wpool.tile([128, len(_W_NAMES) * 128], bf16, tag="w_all")
            nc.sync.dma_start(w_all[:], wts_d[:])
            wt = {
                n: w_all[:, i * 128 : (i + 1) * 128]
                for i, n in enumerate(_W_NAMES)
            }
            # tiny warmup matmul: PE observes the weights DMA here, so no
            # per-tile matmul ever carries that wait (S3_LW wait slots <= 2)
            warm = psum.tile([1, 4], f32, tag="YT")
            with tc.high_priority():
                nc.tensor.matmul(warm[0:1, 0:1], w_all[0:1, 0:1], w_all[0:1, 0:1])

            for _rep in range(reps):
              for ch in range(n_ch):
                for r0, rt in tiles:
                    first = r0 == 0
                    last = r0 + rt == h
                    ka = rt + 1      # A/B/C/XT/YT/product partitions
                    ku = rt + 1 if last else rt + 2  # loaded U partitions
                    # rows to load: 128 where possible (127-partition
                    # DMAs cost ~4x)
                    kld = 128 if ka == 127 else ka
                    # ---- single input load per tile ----
                    T = io.tile([128, row_e], bf16, tag="T")
                    nc.sync.dma_start(T[0:kld, :], uabc_d[ch, r0 : r0 + kld, :])
                    U = T[:, 0:w]
                    ABC = T[:, w:row_e]
                    A = ABC[:, 0 : w + 2]
                    Bt = ABC[:, w + 2 : 2 * (w + 2)]
                    C = ABC[:, 2 * (w + 2) : 3 * (w + 2)]

                    do_xt = mode in ("full", "nope", "nodve", "nomm")
                    do_yt = mode in ("full", "nope", "nodve")
                    do_dve = mode in ("full", "nope", "nomm")
                    do_pe = mode in ("full", "nodve")
                    do_act = mode != "dma"
                    # ---- XT (DVE): free-dim forward diff, col W-1 = 0 ----
                    XT = tmp.tile([128, w], bf16, tag="XT")
                    if do_xt:
                        nc.vector.tensor_sub(
                            XT[0:ka, 0 : w - 1], U[0:ka, 1:w], U[0:ka, 0 : w - 1]
                        )
                        nc.vector.memset(XT[0:ka, w - 1 : w], 0.0)

                    # ---- YT (PE): partition-dim forward diff -> PSUM ----
                    YT = psum.tile([128, w], f32, tag="YT")
                    my = wt[{(0, 0): "my", (1, 0): "myf",
                             (0, 1): "myl", (1, 1): "myfl"}[(first, last)]]
                    if do_yt:
                        for n0 in range(0, w, chunk):
                            nc.tensor.matmul(
                                YT[0:ka, n0 : n0 + chunk],
                                my[0:ku, 0:ka],
                                U[0:ku, n0 : n0 + chunk],
                            )

                    # ---- products (DVE) ----
                    # PA[q, s] = a[r0+q, s] * X[r0+q-1, s-1c]   s in [0, w+1)
                    PA = tmp.tile([128, w + 1], bf16, tag="PA")
                    PB1 = tmp.tile([128, w + 1], bf16, tag="PB1")
                    PB2 = tmp.tile([128, w], bf16, tag="PB2")
                    PC = tmp.tile([128, w], bf16, tag="PC")
                    if do_dve:
                        nc.vector.tensor_mul(
                            PA[0:ka, 1 : w + 1], A[0:ka, 1 : w + 1], XT[0:ka, 0:w]
                        )
                        nc.vector.tensor_mul(PA[0:ka, 0:1], A[0:ka, 0:1], XT[0:ka, 0:1])
                        # PB1[q, s] = b[r0+q, s] * Y[r0+q-1, s-1c]
                        nc.vector.tensor_mul(
                            PB1[0:ka, 1 : w + 1], Bt[0:ka, 1 : w + 1], YT[0:ka, 0:w]
                        )
                        nc.vector.tensor_mul(PB1[0:ka, 0:1], Bt[0:ka, 0:1], YT[0:ka, 0:1])
                        # PB2/PC stored at local col s-1, s in [1, w+1)
                        nc.vector.tensor_mul(
                            PB2[0:ka, 0:w], Bt[0:ka, 1 : w + 1], XT[0:ka, 0:w]
                        )
                        nc.vector.tensor_mul(PC[0:ka, 0:w], C[0:ka, 1 : w + 1], YT[0:ka, 0:w])

                    # ---- PSUM assembly (PE) ----
                    OUTP = psum.tile([128, w], f32, tag="OUTP")
                    for n0 in (range(0, w, chunk) if do_pe else ()):
                        cw = min(chunk, w - n0)
                        o = OUTP[0:rt, n0 : n0 + cw]
                        mm = [
                            (wt["wu"][0:ka, 0:rt], U[0:ka, n0 : n0 + cw]),
                            (wt["wsp"][0:ka, 0:rt], PA[0:ka, n0 + 1 : n0 + 1 + cw]),
                            (wt["wsn"][0:ka, 0:rt], PA[0:ka, n0 : n0 + cw]),
                            (wt["wsp"][0:ka, 0:rt], PB1[0:ka, n0 + 1 : n0 + 1 + cw]),
                            (wt["wsn"][0:ka, 0:rt], PB1[0:ka, n0 : n0 + cw]),
                            (wt["wg"][0:ka, 0:rt], PB2[0:ka, n0 : n0 + cw]),
                            (wt["wg"][0:ka, 0:rt], PC[0:ka, n0 : n0 + cw]),
                        ]
                        for i, (lhsT, rhs) in enumerate(mm):
                            nc.tensor.matmul(
                                o,
                                lhsT,
                                rhs,
                                start=(i == 0),
                                stop=(i == len(mm) - 1),
                            )

                    # ---- PSUM -> SBUF (ACT), store ----
                    OS = tmp.tile([128, w], bf16, tag="OS")
                    if do_act:
                        nc.scalar.copy(OS[0:rt, :], OUTP[0:rt, :])
                    else:
                        nc.vector.memset(OS[0:1, 0:4], 0.0)
                    if do_act and not do_pe:
                        nc.vector.memset(OUTP[0:1, 0:4], 0.0)
                    if do_dve and not do_yt:
                        nc.vector.memset(YT[0:1, 0:4], 0.0)
                    if do_pe and not do_dve:
                        for _t in (PA, PB1, PB2, PC):
                            nc.vector.memset(_t[0:1, 0:4], 0.0)
                    nc.sync.dma_start(out_d[ch, r0 : r0 + rt, :], OS[0:rt, :])

    nc.compile()
    return nc


def _cast_inputs(u, a, b, c):
    """Pack uabc[n, ch, H+2, W + 3*(W+2)] bf16: per padded row r, u row
    clamp(r-1) then a/b/c rows r."""
    import ml_dtypes

    bf = ml_dtypes.bfloat16
    u_bf = np.asarray(u, dtype=np.float32).astype(bf)
    rows = np.clip(np.arange(u_bf.shape[2] + 2) - 1, 0, u_bf.shape[2] - 1)
    u_ext = u_bf[:, :, rows, :]  # [n, ch, H+2, W]
    abc = np.stack(
        [np.asarray(t, dtype=np.float32).astype(bf) for t in (a, b, c)], axis=3
    )  # [n, ch, H+2, 3, W+2]
    abc = abc.reshape(*abc.shape[:3], -1)
    uabc = np.ascontiguousarray(np.concatenate([u_ext, abc], axis=3))
    return uabc


def kernel(u, a, b, c, tau, grad_x, grad_y):
    from concourse.bass_utils import run_bass_kernel_spmd

    uabc = _cast_inputs(u, a, b, c)
    hx = float(np.asarray(grad_x)[0, 0, 1, 2])
    s = float(np.asarray(tau)) * hx * hx
    rt_last = H % R if H % R else R
    wts = _host_weights(s, rt_last)

    nc = _build_nc(N_CH, H, W, R, CHUNK)
    in_maps = [
        {"uabc": uabc[k], "wts": wts}
        for k in range(N_CORES)
    ]
    res = run_bass_kernel_spmd(nc, in_maps, list(range(N_CORES)))
    return np.stack(
        [res.results[k]["out"].astype(np.float32) for k in range(N_CORES)], axis=0
    )
